# revision 1
# baseline (speedup 1.0000x reference)
"""Trainium2 Bass kernel for AttnProcessor self-attention (B=2,S=2048,C=1024,H=16).

Sharding: 8 cores, core c owns heads (2c, 2c+1) for both batches (tensor
parallel on the head dim for QKV); attention outputs are redistributed with
two 8-core AllToAlls (one per batch, the first hidden under batch-1 compute)
so core c computes the output projection + residual for output rows
(b=c//4, s in [512*(c%4), 512*(c%4+1))). Host picks out1/out2 per core.

Per-core pipeline (all matmuls fp32r):
  qT/kT projections in [c'=128, s] layout, v' in [s, 2x(64+ones)] layout
  (padded to 256 free for fp32r full rate), row-tiled (64x128) QK^T per head
  pair, exp on ScalarE (scale=1/8, no max subtraction -- scores are O(5)),
  PV accumulates V'.T @ probsT giving both the attention output (transposed)
  and the softmax denominators (ones row). Denominator reciprocals are
  computed batched pre-collective and ride the AllToAll; normalization +
  output projection run per received chunk.
"""
import numpy as np

import concourse.bacc as bacc
import concourse.bass as bass
import concourse.tile as tile
import concourse.tile_rust as tile_rust
from concourse import mybir
from concourse.bass_utils import run_bass_kernel_spmd

F32 = mybir.dt.float32
F32R = mybir.dt.float32r

B, S, C, H, D = 2, 2048, 1024, 16, 64
N_CORES = 8
BS = B * S  # 4096
SCALE = 1.0 / np.sqrt(D)

_CACHE = {}


def _build():
    nc = bacc.Bacc(num_devices=N_CORES)
    hsT = nc.declare_dram_parameter("hsT", [C, BS], F32R, isOutput=False)
    wq = nc.declare_dram_parameter("wq", [C, 128], F32R, isOutput=False)
    wk = nc.declare_dram_parameter("wk", [C, 128], F32R, isOutput=False)
    wv = nc.declare_dram_parameter("wv", [C, 256], F32R, isOutput=False)
    wo = nc.declare_dram_parameter("wo", [C, C], F32R, isOutput=False)
    bqk = nc.declare_dram_parameter("bqk", [128, 2], F32, isOutput=False)
    bvb = nc.declare_dram_parameter("bvb", [1, 256], F32, isOutput=False)
    res = nc.declare_dram_parameter("res", [512, C], F32, isOutput=False)
    out1 = nc.declare_dram_parameter("out1", [256, C], F32, isOutput=True)
    out2 = nc.declare_dram_parameter("out2", [256, C], F32, isOutput=True)

    with tile.TileContext(nc) as tc:
        with (
            tc.tile_pool(name="wpool", bufs=1) as wpool,
            tc.tile_pool(name="hpool", bufs=1) as hpool,
            tc.tile_pool(name="qkpool", bufs=2) as qkpool,
            tc.tile_pool(name="ppool", bufs=3) as ppool,
            tc.tile_pool(name="spool", bufs=3) as spool,
            tc.tile_pool(name="opool", bufs=2) as opool,
            tc.tile_pool(name="psum", bufs=1, space="PSUM") as psum,
            tc.tile_pool(name="dram", bufs=1, space="DRAM") as dram,
        ):
            # ---- weight / constant / input loads ----
            # single strided DMA per weight tensor: sbuf [128, 8*N] with
            # chunk cc at cols N*cc  <-  dram [1024, N]
            wo_sb = []

            def load_w(name, src, ncols):
                t = wpool.tile([128, 8 * ncols], F32R, tag=name)
                sap = src[:]
                nc.scalar.dma_start(
                    out=t[:],
                    in_=bass.AP(tensor=sap.tensor, offset=sap.offset,
                                ap=[[ncols, 128], [128 * ncols, 8],
                                    [1, ncols]]))
                return [t[:, ncols * cc:ncols * (cc + 1)] for cc in range(8)]

            wq_sb = load_w("wq", wq, 128)
            hs0 = []
            for cc in range(8):
                t = hpool.tile([128, 2048], F32R, tag=f"hs{cc}", name=f"hs0_{cc}")
                hs0.append(t)
            for g in range(2):
                for cc in range(8):
                    eng = nc.scalar if g == 0 else nc.sync
                    eng.dma_start(
                        out=hs0[cc][:, 1024 * g:1024 * (g + 1)],
                        in_=hsT[128 * cc:128 * (cc + 1),
                                1024 * g:1024 * (g + 1)])
            wk_sb = load_w("wk", wk, 128)
            wv_sb = load_w("wv", wv, 256)
            bqk_sb = wpool.tile([128, 2], F32, tag="bqk")
            nc.scalar.dma_start(out=bqk_sb[:], in_=bqk[:])
            bvb_sb = wpool.tile([128, 256], F32, tag="bvb")
            bvb_ap = bvb[:]
            nc.scalar.dma_start(
                out=bvb_sb[:],
                in_=bass.AP(tensor=bvb_ap.tensor, offset=bvb_ap.offset,
                            ap=[[0, 128], [1, 256]]),
            )

            a2a_in = [dram.tile([8, 130, 256], F32R, name=f"a2ain{b}")
                      for b in range(2)]
            a2a_out = [dram.tile([8, 130, 256], F32R, name=f"a2aout{b}")
                       for b in range(2)]

            qT, kT, vS, sums_pre = {}, {}, {}, {}
            last_drain = [None]

            def emit_hsT_load(b):
                tiles = []
                for cc in range(8):
                    t = hpool.tile([128, 2048], F32R, tag=f"hs{cc}",
                                   name=f"hs{b}_{cc}")
                    nc.scalar.dma_start(
                        out=t[:],
                        in_=hsT[128 * cc:128 * (cc + 1), 2048 * b:2048 * (b + 1)])
                    tiles.append(t)
                return tiles

            def emit_proj_qk(b, hs_sb, t_idx, j):
                """One unit: tensor t_idx (0=q,1=k), one 512-wide s-slice j."""
                if t_idx == 0:
                    if b not in qT:
                        qT[b] = qkpool.tile([128, 2048], F32R, tag="qT",
                                            name=f"qT{b}")
                    dst, w_sb = qT[b], wq_sb
                else:
                    if b not in kT:
                        kT[b] = qkpool.tile([128, 2048], F32R, tag="kT",
                                            name=f"kT{b}")
                    dst, w_sb = kT[b], wk_sb
                ps = psum.tile([128, 512], F32, tag="big", bufs=3,
                               name=f"pqk{b}_{t_idx}_{j}")
                for cc in range(8):
                    nc.tensor.matmul(
                        ps[:], w_sb[cc],
                        hs_sb[cc][:, 512 * j:512 * (j + 1)],
                        start=(cc == 0), stop=(cc == 7))
                nc.vector.tensor_scalar_add(
                    out=dst[:, 512 * j:512 * (j + 1)], in0=ps[:],
                    scalar1=bqk_sb[:, t_idx:t_idx + 1])

            def emit_proj_v(b, hs_sb, i):
                """One unit: one 128-row v' s-tile i."""
                if b not in vS:
                    vS[b] = qkpool.tile([128, 2080], F32R, tag="vS",
                                        name=f"vS{b}")
                dst = vS[b]
                ps = psum.tile([128, 512], F32, tag="big", bufs=3,
                               name=f"pv{b}_{i}")
                sl = ps[:, 0:256]
                for cc in range(8):
                    nc.tensor.matmul(
                        sl, hs_sb[cc][:, 128 * i:128 * (i + 1)], wv_sb[cc],
                        start=(cc == 0), stop=(cc == 7))
                nc.vector.tensor_tensor(
                    out=dst[:, 130 * i:130 * (i + 1)], in0=sl[:, 0:130],
                    in1=bvb_sb[:, 0:130], op=mybir.AluOpType.add)

            def emit_attention_qs(b, qs, fill_work):
                """One q-slice (512 q) for both heads, processed in kc-pairs:
                per step, fills then 2 exps, then 4 QK mms (64-row config),
                then 4 PV mms (128-row config, bank-paired A,A,B,B)."""
                accA = psum.tile([65, 512], F32, tag="accA", bufs=1,
                                 name=f"accA_{b}_{qs}")
                accB = psum.tile([65, 512], F32, tag="accB", bufs=1,
                                 name=f"accB_{b}_{qs}")
                sc_t = {}

                def emit_qk(kc):
                    sc = psum.tile([128, 1024], F32, tag="big", bufs=3,
                                   name=f"sc_{b}_{qs}_{kc}")
                    sc_t[kc] = sc
                    nc.tensor.matmul(
                        sc[:, 0:512],
                        kT[b][0:64, 128 * kc:128 * (kc + 1)],
                        qT[b][0:64, 512 * qs:512 * (qs + 1)],
                        start=True, stop=True)
                    nc.tensor.matmul(
                        sc[:, 512:1024],
                        kT[b][64:128, 128 * kc:128 * (kc + 1)],
                        qT[b][64:128, 512 * qs:512 * (qs + 1)],
                        start=True, stop=True)

                def emit_pv(acc, off, kc, pr):
                    nc.tensor.matmul(
                        acc[:],
                        vS[b][:, 130 * kc + off:130 * kc + off + 65],
                        pr[:, (0 if off == 0 else 512):
                           (512 if off == 0 else 1024)],
                        start=(kc == 0), stop=(kc == 15))

                emit_qk(0)
                emit_qk(1)
                for step in range(8):
                    kc0, kc1 = 2 * step, 2 * step + 1
                    for _ in range(2):
                        if fill_work:
                            fill_work.pop(0)()
                    pr0 = ppool.tile([128, 1024], F32R, tag="pr", bufs=4,
                                     name=f"pr_{b}_{qs}_{kc0}")
                    nc.scalar.activation(pr0[:], sc_t.pop(kc0)[:],
                                         mybir.ActivationFunctionType.Exp,
                                         scale=float(SCALE))
                    pr1 = ppool.tile([128, 1024], F32R, tag="pr", bufs=4,
                                     name=f"pr_{b}_{qs}_{kc1}")
                    nc.scalar.activation(pr1[:], sc_t.pop(kc1)[:],
                                         mybir.ActivationFunctionType.Exp,
                                         scale=float(SCALE))
                    if step < 7:
                        emit_qk(kc0 + 2)
                        emit_qk(kc1 + 2)
                    emit_pv(accA, 0, kc0, pr0)
                    emit_pv(accA, 0, kc1, pr1)
                    emit_pv(accB, 65, kc0, pr0)
                    emit_pv(accB, 65, kc1, pr1)
                # drain: rows [0:64] -> a2a_in, row 64 (sums) -> sums_pre
                if b not in sums_pre:
                    sums_pre[b] = opool.tile([8, 512], F32R, tag="sums",
                                             name=f"sums{b}")
                for h, acc in ((0, accA), (1, accB)):
                    st = spool.tile([65, 512], F32R, tag="st",
                                    name=f"st_{b}_{qs}_{h}")
                    nc.vector.tensor_copy(st[:], acc[:])
                    for half in range(2):
                        d = nc.sync.dma_start(
                            out=a2a_in[b][2 * qs + half,
                                          64 * h:64 * (h + 1), :],
                            in_=st[0:64, 256 * half:256 * (half + 1)])
                        last_drain[0] = d
                    nc.sync.dma_start(
                        out=sums_pre[b][2 * qs + h:2 * qs + h + 1, :],
                        in_=st[64:65, :])

            def emit_recip_ship(b):
                with nc.allow_low_precision("f32r softmax denominators"):
                    nc.vector.reciprocal(sums_pre[b][:],
                                         sums_pre[b][:].bitcast(F32))
                for qs in range(4):
                    for h in range(2):
                        for half in range(2):
                            nc.sync.dma_start(
                                out=a2a_in[b][2 * qs + half,
                                              128 + h:129 + h, :],
                                in_=sums_pre[b][2 * qs + h:2 * qs + h + 1,
                                                256 * half:256 * (half + 1)])

            def emit_collective(b):
                nc.gpsimd.collective_compute(
                    "AllToAll", mybir.AluOpType.bypass,
                    replica_groups=[list(range(8))],
                    ins=[a2a_in[b][:]], outs=[a2a_out[b][:]])

            def emit_output(b, out_t, res_pair, after=None):
                """Normalize received [128,256] chunks, outproj 2 stiles."""
                op_ps = []
                for st_i in range(2):
                    ps = psum.tile([128, 1024], F32, tag="big", bufs=3,
                                   name=f"op{b}_{st_i}")
                    op_ps.append((ps[:, 0:512], ps[:, 512:1024], ps))
                for j in range(8):
                    raw = opool.tile([128, 256], F32, tag="raw",
                                     name=f"raw{b}_{j}")
                    rd = nc.scalar.dma_start(out=raw[:],
                                             in_=a2a_out[b][j, 0:128, :]
                                             .bitcast(F32))
                    if after is not None and j == 0:
                        tile_rust.add_dep_helper(
                            rd.ins, after.ins, False,
                            "hold output norm until attention drained")
                    rbc = opool.tile([128, 256], F32, tag="rbc",
                                     name=f"rbc{b}_{j}")
                    for h in range(2):
                        srow = a2a_out[b][j, 128 + h:129 + h, :].bitcast(F32)
                        nc.scalar.dma_start(
                            out=rbc[64 * h:64 * (h + 1), :],
                            in_=bass.AP(tensor=srow.tensor, offset=srow.offset,
                                        ap=[[0, 64], [1, 256]]))
                    an_t = opool.tile([128, 256], F32R, tag="an",
                                      name=f"an{b}_{j}")
                    an = an_t[:]
                    nc.vector.tensor_tensor(out=an, in0=raw[:], in1=rbc[:],
                                            op=mybir.AluOpType.mult)
                    for st_i in range(2):
                        for co in range(2):
                            nc.tensor.matmul(
                                op_ps[st_i][co],
                                an[:, 128 * st_i:128 * (st_i + 1)],
                                wo_sb[j][:, 512 * co:512 * (co + 1)],
                                start=(j == 0), stop=(j == 7))
                for st_i in range(2):
                    ob = opool.tile([128, 1024], F32, tag="ob",
                                    name=f"ob{b}_{st_i}")
                    nc.vector.tensor_tensor(out=ob[:], in0=op_ps[st_i][2][:],
                                            in1=res_pair[st_i][:],
                                            op=mybir.AluOpType.add)
                    nc.sync.dma_start(
                        out=out_t[128 * st_i:128 * (st_i + 1), 0:512],
                        in_=ob[:, 0:512])
                    nc.scalar.dma_start(
                        out=out_t[128 * st_i:128 * (st_i + 1), 512:1024],
                        in_=ob[:, 512:1024])

            # ---------------- emission ----------------
            # prefix: just enough b0 projection for attention(b0, qs0) kc 0-3
            emit_proj_qk(0, hs0, 0, 0)
            emit_proj_qk(0, hs0, 1, 0)
            for i in range(4):
                emit_proj_v(0, hs0, i)

            hs1 = emit_hsT_load(1)

            def qk_u(b, hs, t, j):
                return lambda: emit_proj_qk(b, hs, t, j)

            def v_u(b, hs, i):
                return lambda: emit_proj_v(b, hs, i)

            # qs0 fill: 2 pops per step-start; each unit lands before its
            # first consumer (vS stile i -> PV at step i//2; kT unit j ->
            # QK(4j) emitted at step 2j-1; deadlines checked offline)
            fill = [qk_u(0, hs0, 1, 1), v_u(0, hs0, 4), v_u(0, hs0, 5),
                    qk_u(0, hs0, 1, 2), v_u(0, hs0, 6), v_u(0, hs0, 7),
                    v_u(0, hs0, 8), v_u(0, hs0, 9), qk_u(0, hs0, 1, 3),
                    v_u(0, hs0, 10), v_u(0, hs0, 11), v_u(0, hs0, 12),
                    v_u(0, hs0, 13), v_u(0, hs0, 14), v_u(0, hs0, 15),
                    qk_u(0, hs0, 0, 1)]
            emit_attention_qs(0, 0, fill)
            fill = [qk_u(0, hs0, 0, 2), qk_u(0, hs0, 0, 3)]
            emit_attention_qs(0, 1, fill)
            fill = []
            for t_idx in range(2):
                for j in range(4):
                    if t_idx == 0 and j >= 2:
                        continue  # deferred into attention(b1) slack
                    fill.append(qk_u(1, hs1, t_idx, j))
            for i in range(16):
                fill.append(v_u(1, hs1, i))
            emit_attention_qs(0, 2, fill)
            emit_attention_qs(0, 3, fill)
            while fill:
                fill.pop(0)()
            emit_recip_ship(0)
            emit_collective(0)

            # load wo / res during attention(b1); reuse freed slots
            for cc in range(8):
                t = hpool.tile([128, 1024], F32R, tag=f"hs{cc}",
                               name=f"wo{cc}")
                nc.sync.dma_start(out=t[:], in_=wo[128 * cc:128 * (cc + 1), :])
                wo_sb.append(t)
            res_sb = []
            for st_i in range(4):
                t = wpool.tile([128, 1024], F32, tag=f"res{st_i}",
                               name=f"res{st_i}")
                nc.sync.dma_start(out=t[:],
                                  in_=res[128 * st_i:128 * (st_i + 1), :])
                res_sb.append(t)

            fill_b1 = [qk_u(1, hs1, 0, 2), qk_u(1, hs1, 0, 3)]
            for qs in range(4):
                emit_attention_qs(1, qs, fill_b1)
            emit_recip_ship(1)
            emit_output(0, out1, res_sb[0:2], after=last_drain[0])
            emit_collective(1)
            emit_output(1, out2, res_sb[2:4])
    nc.finalize()
    return nc


def _prep_inputs(hidden_states, Wq, bq, Wk, bk, Wv, bv, Wo, bo):
    hs = np.asarray(hidden_states, np.float32)
    hsT = np.ascontiguousarray(
        hs.transpose(2, 0, 1).reshape(C, BS)).astype(np.float32)
    Wo_f = np.ascontiguousarray(np.asarray(Wo, np.float32))
    in_maps = []
    for c in range(N_CORES):
        h0 = 2 * c
        cols = slice(64 * h0, 64 * h0 + 128)
        wv_c = np.zeros((C, 256), np.float32)
        bvb_c = np.zeros((1, 256), np.float32)
        for a in range(2):
            hd = slice(64 * (h0 + a), 64 * (h0 + a + 1))
            wv_c[:, 65 * a:65 * a + 64] = np.asarray(Wv, np.float32)[:, hd]
            bvb_c[0, 65 * a:65 * a + 64] = np.asarray(bv, np.float32)[hd]
            bvb_c[0, 65 * a + 64] = 1.0
        bqk_c = np.stack([np.asarray(bq, np.float32)[cols],
                          np.asarray(bk, np.float32)[cols]], axis=1)
        s0 = 256 * c
        bo_f = np.asarray(bo, np.float32)
        res_c = np.concatenate(
            [hs[0, s0:s0 + 256, :] + bo_f, hs[1, s0:s0 + 256, :] + bo_f],
            axis=0).astype(np.float32)
        in_maps.append({
            "hsT": hsT,
            "wq": np.ascontiguousarray(np.asarray(Wq, np.float32)[:, cols]),
            "wk": np.ascontiguousarray(np.asarray(Wk, np.float32)[:, cols]),
            "wv": wv_c,
            "wo": Wo_f,
            "bqk": np.ascontiguousarray(bqk_c),
            "bvb": bvb_c,
            "res": np.ascontiguousarray(res_c),
        })
    return in_maps


def _run(inputs, trace=False, trace_kwargs=None):
    if "nc" not in _CACHE:
        _CACHE["nc"] = _build()
    nc = _CACHE["nc"]
    in_maps = _prep_inputs(**inputs)
    r = run_bass_kernel_spmd(nc, in_maps, core_ids=list(range(N_CORES)),
                             trace=trace, **(trace_kwargs or {}))
    full = np.empty((B, S, C), np.float32)
    for c in range(N_CORES):
        full[0, 256 * c:256 * (c + 1), :] = r.results[c]["out1"]
        full[1, 256 * c:256 * (c + 1), :] = r.results[c]["out2"]
    return full, r


def kernel(**inputs):
    full, _ = _run(inputs, trace=False)
    return full



# revision 11
# speedup vs baseline: 1.1775x; 1.1775x over previous
"""Trainium2 Bass kernel for AttnProcessor self-attention (B=2,S=2048,C=1024,H=16).

Sharding: 8 cores, core c owns heads (2c, 2c+1) for both batches (tensor
parallel on the head dim for QKV); attention outputs are redistributed with
two 8-core AllToAlls (one per batch; both hidden under compute) so core c
computes the output projection + residual for output rows
(b, s in [256*c, 256*(c+1))). Host picks out1/out2 per core.

v2: all-bf16 compute (inputs/weights/probs/payload), f32 PSUM accumulate.
ScalarE runs exp exclusively; every DMA issues from sync/vector/gpsimd/
tensor queues. Both batches' hidden states + all weights stream in at t=0.
Output projection for batch 0 is interleaved into batch-1 attention as fill
work; softmax reciprocals ride the AllToAll (rows 128-129) and are
broadcast on the receiver with a tiny selector matmul on the idle PE.
"""
import numpy as np
import ml_dtypes

import concourse.bacc as bacc
import concourse.bass as bass
import concourse.tile as tile
from concourse import mybir
from concourse.bass_utils import run_bass_kernel_spmd

F32 = mybir.dt.float32
BF16 = mybir.dt.bfloat16

B, S, C, H, D = 2, 2048, 1024, 16, 64
N_CORES = 8
BS = B * S  # 4096
SCALE = 1.0 / np.sqrt(D)
SLAB = 130 * 256  # a2a slab stride

_CACHE = {}


def _build():
    nc = bacc.Bacc(num_devices=N_CORES)
    hsT = nc.declare_dram_parameter("hsT", [C, BS], BF16, isOutput=False)
    wq = nc.declare_dram_parameter("wq", [C, 128], BF16, isOutput=False)
    wk = nc.declare_dram_parameter("wk", [C, 128], BF16, isOutput=False)
    wv = nc.declare_dram_parameter("wv", [C, 130], BF16, isOutput=False)
    wo = nc.declare_dram_parameter("wo", [C, C], BF16, isOutput=False)
    bqk = nc.declare_dram_parameter("bqk", [128, 2], F32, isOutput=False)
    bvb = nc.declare_dram_parameter("bvb", [1, 130], F32, isOutput=False)
    res = nc.declare_dram_parameter("res", [512, C], F32, isOutput=False)
    eselp = nc.declare_dram_parameter("esel", [2, 128], BF16, isOutput=False)
    out1 = nc.declare_dram_parameter("out1", [256, C], F32, isOutput=True)
    out2 = nc.declare_dram_parameter("out2", [256, C], F32, isOutput=True)

    with tile.TileContext(nc) as tc:
        with (
            tc.tile_pool(name="wpool", bufs=1) as wpool,
            tc.tile_pool(name="hpool", bufs=1) as hpool,
            tc.tile_pool(name="qkpool", bufs=2) as qkpool,
            tc.tile_pool(name="ppool", bufs=4) as ppool,
            tc.tile_pool(name="spool", bufs=3) as spool,
            tc.tile_pool(name="opool", bufs=2) as opool,
            tc.tile_pool(name="psum", bufs=1, space="PSUM") as psum,
            tc.tile_pool(name="dram", bufs=1, space="DRAM") as dram,
        ):
            rr = [nc.sync, nc.gpsimd]

            # ---- weight loads: single strided DMA per tensor ----
            def load_w(name, src, ncols, eng):
                t = wpool.tile([128, 8 * ncols], BF16, tag=name)
                sap = src[:]
                eng.dma_start(
                    out=t[:],
                    in_=bass.AP(tensor=sap.tensor, offset=sap.offset,
                                ap=[[ncols, 128], [128 * ncols, 8],
                                    [1, ncols]]))
                return [t[:, ncols * cc:ncols * (cc + 1)] for cc in range(8)]

            wq_sb = load_w("wq", wq, 128, nc.sync)
            wk_sb = load_w("wk", wk, 128, nc.gpsimd)

            # hs tiles, both batches resident; b0 halves h=0 first
            hs = [[], []]
            for b in range(2):
                for cc in range(8):
                    t = hpool.tile([128, 2048], BF16, tag=f"hs{b}_{cc}",
                                   name=f"hs{b}_{cc}")
                    hs[b].append(t)
            for cc in range(8):
                rr[cc % 2].dma_start(
                    out=hs[0][cc][:, 0:1024],
                    in_=hsT[128 * cc:128 * (cc + 1), 0:1024])
            wv_sb = load_w("wv", wv, 130, nc.sync)
            bqk_sb = wpool.tile([128, 2], F32, tag="bqk")
            nc.gpsimd.dma_start(out=bqk_sb[:], in_=bqk[:])
            bvb_sb = wpool.tile([128, 130], F32, tag="bvb")
            bvb_ap = bvb[:]
            nc.gpsimd.dma_start(
                out=bvb_sb[:],
                in_=bass.AP(tensor=bvb_ap.tensor, offset=bvb_ap.offset,
                            ap=[[0, 128], [1, 130]]))
            for cc in range(8):
                rr[cc % 2].dma_start(
                    out=hs[0][cc][:, 1024:2048],
                    in_=hsT[128 * cc:128 * (cc + 1), 1024:2048])
            for cc in range(8):
                rr[cc % 2].dma_start(
                    out=hs[1][cc][:],
                    in_=hsT[128 * cc:128 * (cc + 1), 2048:4096])
            wo_sb = []
            for cc in range(8):
                t = wpool.tile([128, 1024], BF16, tag=f"wo{cc}",
                               name=f"wo{cc}")
                rr[cc % 2].dma_start(out=t[:],
                                     in_=wo[128 * cc:128 * (cc + 1), :])
                wo_sb.append(t)
            res_sb = []
            for st_i in range(4):
                t = wpool.tile([128, 1024], F32, tag=f"res{st_i}",
                               name=f"res{st_i}")
                rr[st_i % 2].dma_start(
                    out=t[:], in_=res[128 * st_i:128 * (st_i + 1), :])
                res_sb.append(t)
            # head-selector for reciprocal broadcast: esel[h, m] = (m//64==h)
            esel = wpool.tile([2, 128], BF16, tag="esel")
            nc.gpsimd.dma_start(out=esel[:], in_=eselp[:])

            a2a_in = [dram.tile([8, 130, 256], BF16, name=f"a2ain{b}")
                      for b in range(2)]
            a2a_out = [dram.tile([8, 130, 256], BF16, name=f"a2aout{b}")
                       for b in range(2)]

            qT, kT, vS, sums_pre = {}, {}, {}, {}

            def emit_proj_qk(b, t_idx, j, pool, tag):
                """One unit: tensor t_idx (0=q,1=k), one 512-wide s-slice j."""
                if t_idx == 0:
                    if b not in qT:
                        qT[b] = qkpool.tile([128, 2048], BF16, tag="qT",
                                            name=f"qT{b}")
                    dst, w_sb = qT[b], wq_sb
                else:
                    if b not in kT:
                        kT[b] = qkpool.tile([128, 2048], BF16, tag="kT",
                                            name=f"kT{b}")
                    dst, w_sb = kT[b], wk_sb
                ps = psum.tile([128, 1024], F32, tag=tag, bufs=pool,
                               name=f"pqk{b}_{t_idx}_{j}")
                sl = ps[:, 0:512]
                for cc in range(8):
                    nc.tensor.matmul(
                        sl, w_sb[cc],
                        hs[b][cc][:, 512 * j:512 * (j + 1)],
                        start=(cc == 0), stop=(cc == 7))
                nc.vector.tensor_scalar_add(
                    out=dst[:, 512 * j:512 * (j + 1)], in0=sl,
                    scalar1=bqk_sb[:, t_idx:t_idx + 1])

            def emit_proj_v(b, i, pool, tag):
                """One unit: one 128-row v' s-tile i."""
                if b not in vS:
                    vS[b] = qkpool.tile([128, 2080], BF16, tag="vS",
                                        name=f"vS{b}")
                dst = vS[b]
                ps = psum.tile([128, 1024], F32, tag=tag, bufs=pool,
                               name=f"pv{b}_{i}")
                sl = ps[:, 0:130]
                for cc in range(8):
                    nc.tensor.matmul(
                        sl, hs[b][cc][:, 128 * i:128 * (i + 1)], wv_sb[cc],
                        start=(cc == 0), stop=(cc == 7))
                nc.vector.tensor_tensor(
                    out=dst[:, 130 * i:130 * (i + 1)], in0=sl,
                    in1=bvb_sb[:, 0:130], op=mybir.AluOpType.add)

            def emit_attention_qs(b, qs, fill_work):
                """One q-slice (512 q) for both heads, kc-pair steps."""
                accA = psum.tile([65, 512], F32, tag="accA", bufs=1,
                                 name=f"accA_{b}_{qs}")
                accB = psum.tile([65, 512], F32, tag="accB", bufs=1,
                                 name=f"accB_{b}_{qs}")
                sc_t = {}

                def emit_qk(kc):
                    sc = psum.tile([128, 1024], F32, tag="big", bufs=2,
                                   name=f"sc_{b}_{qs}_{kc}")
                    sc_t[kc] = sc
                    nc.tensor.matmul(
                        sc[:, 0:512],
                        kT[b][0:64, 128 * kc:128 * (kc + 1)],
                        qT[b][0:64, 512 * qs:512 * (qs + 1)],
                        start=True, stop=True)
                    nc.tensor.matmul(
                        sc[:, 512:1024],
                        kT[b][64:128, 128 * kc:128 * (kc + 1)],
                        qT[b][64:128, 512 * qs:512 * (qs + 1)],
                        start=True, stop=True)

                def emit_pv(acc, off, kc, pr):
                    nc.tensor.matmul(
                        acc[:],
                        vS[b][:, 130 * kc + off:130 * kc + off + 65],
                        pr[:, (0 if off == 0 else 512):
                           (512 if off == 0 else 1024)],
                        start=(kc == 0), stop=(kc == 15))

                emit_qk(0)
                emit_qk(1)
                for step in range(8):
                    kc0, kc1 = 2 * step, 2 * step + 1
                    for _ in range(2):
                        if fill_work:
                            fill_work.pop(0)()
                    pr0 = ppool.tile([128, 1024], BF16, tag="pr", bufs=4,
                                     name=f"pr_{b}_{qs}_{kc0}")
                    nc.scalar.activation(pr0[:], sc_t.pop(kc0)[:],
                                         mybir.ActivationFunctionType.Exp,
                                         scale=float(SCALE))
                    pr1 = ppool.tile([128, 1024], BF16, tag="pr", bufs=4,
                                     name=f"pr_{b}_{qs}_{kc1}")
                    nc.scalar.activation(pr1[:], sc_t.pop(kc1)[:],
                                         mybir.ActivationFunctionType.Exp,
                                         scale=float(SCALE))
                    if step < 7:
                        emit_qk(kc0 + 2)
                        emit_qk(kc1 + 2)
                    emit_pv(accA, 0, kc0, pr0)
                    emit_pv(accA, 0, kc1, pr1)
                    emit_pv(accB, 65, kc0, pr0)
                    emit_pv(accB, 65, kc1, pr1)
                # drain: rows [0:64] -> a2a_in slabs, row 64 -> sums_pre
                if b not in sums_pre:
                    sums_pre[b] = spool.tile([8, 512], BF16, tag="sums",
                                             name=f"sums{b}")
                for h, acc in ((0, accA), (1, accB)):
                    st = spool.tile([65, 512], BF16, tag="st",
                                    name=f"st_{b}_{qs}_{h}")
                    nc.vector.tensor_copy(st[:], acc[:])
                    dst = a2a_in[b][:]
                    nc.sync.dma_start(
                        out=bass.AP(
                            tensor=dst.tensor,
                            offset=(dst.offset + (2 * qs) * SLAB
                                    + 64 * h * 256),
                            ap=[[256, 64], [SLAB, 2], [1, 256]]),
                        in_=st[0:64, :])
                    nc.gpsimd.dma_start(
                        out=sums_pre[b][2 * qs + h:2 * qs + h + 1, :],
                        in_=st[64:65, :])

            def emit_recip_ship(b):
                with nc.allow_low_precision("bf16 softmax denominators"):
                    nc.vector.reciprocal(sums_pre[b][:], sums_pre[b][:])
                for j in range(8):
                    qs, half = j // 2, j % 2
                    dst = a2a_in[b][:]
                    nc.gpsimd.dma_start(
                        out=bass.AP(
                            tensor=dst.tensor,
                            offset=dst.offset + j * SLAB + 128 * 256,
                            ap=[[256, 2], [1, 256]]),
                        in_=sums_pre[b][2 * qs:2 * qs + 2,
                                        256 * half:256 * half + 256])

            def emit_collective(b):
                nc.gpsimd.collective_compute(
                    "AllToAll", mybir.AluOpType.bypass,
                    replica_groups=[list(range(8))],
                    ins=[a2a_in[b][:]], outs=[a2a_out[b][:]])

            def emit_output_units(b, out_t, res_pair, pool, tag):
                """Returns list of units: bulk load, recip bcast+normalize,
                out-projection (2 q-halves), residual+store."""
                state = {}

                def u_load():
                    raw = opool.tile([128, 2048], BF16, tag="raw",
                                     name=f"raw{b}")
                    rap = a2a_out[b][:]
                    nc.sync.dma_start(
                        out=raw[:],
                        in_=bass.AP(tensor=rap.tensor, offset=rap.offset,
                                    ap=[[256, 128], [SLAB, 8], [1, 256]]))
                    rc = opool.tile([2, 2048], BF16, tag="rcp",
                                    name=f"rcp{b}")
                    nc.gpsimd.dma_start(
                        out=rc[:],
                        in_=bass.AP(tensor=rap.tensor,
                                    offset=rap.offset + 128 * 256,
                                    ap=[[256, 2], [SLAB, 8], [1, 256]]))
                    state["raw"], state["rc"] = raw, rc
                    state["an"] = opool.tile([128, 2048], BF16, tag="an",
                                             name=f"an{b}")

                def u_norm(g):
                    def f():
                        bc = psum.tile([128, 1024], F32, tag=tag, bufs=pool,
                                       name=f"bc{b}_{g}")
                        for hf in range(2):
                            nc.tensor.matmul(
                                bc[:, 512 * hf:512 * (hf + 1)], esel[:],
                                state["rc"][:, 1024 * g + 512 * hf:
                                            1024 * g + 512 * (hf + 1)],
                                start=True, stop=True)
                        nc.vector.tensor_tensor(
                            out=state["an"][:, 1024 * g:1024 * (g + 1)],
                            in0=state["raw"][:, 1024 * g:1024 * (g + 1)],
                            in1=bc[:], op=mybir.AluOpType.mult)
                    return f

                def u_proj(st_i, jlo, jhi, first):
                    def f():
                        if first:
                            state[f"op{st_i}"] = psum.tile(
                                [128, 1024], F32, tag=tag, bufs=pool,
                                name=f"op{b}_{st_i}")
                        ps = state[f"op{st_i}"]
                        an = state["an"]
                        for j in range(jlo, jhi):
                            for co in range(2):
                                nc.tensor.matmul(
                                    ps[:, 512 * co:512 * (co + 1)],
                                    an[:, 256 * j + 128 * st_i:
                                       256 * j + 128 * st_i + 128],
                                    wo_sb[j][:, 512 * co:512 * (co + 1)],
                                    start=(j == 0), stop=(j == 7))
                    return f

                def u_store(st_i):
                    def f():
                        ob = opool.tile([128, 1024], F32, tag="ob",
                                        name=f"ob{b}_{st_i}")
                        nc.vector.tensor_tensor(
                            out=ob[:], in0=state[f"op{st_i}"][:],
                            in1=res_pair[st_i][:], op=mybir.AluOpType.add)
                        nc.sync.dma_start(
                            out=out_t[128 * st_i:128 * (st_i + 1), :],
                            in_=ob[:])
                    return f

                return [u_load, u_norm(0), u_norm(1),
                        u_proj(0, 0, 4, True), u_proj(0, 4, 8, False),
                        u_store(0),
                        u_proj(1, 0, 4, True), u_proj(1, 4, 8, False),
                        u_store(1)]

            # ---------------- emission ----------------
            def qk_u(b, t, j, pool=1, tag="aux"):
                return lambda: emit_proj_qk(b, t, j, pool, tag)

            def v_u(b, i, pool=1, tag="aux"):
                return lambda: emit_proj_v(b, i, pool, tag)

            # prefix: just enough b0 projection for attention(b0, qs0) kc 0-3
            emit_proj_qk(0, 0, 0, 2, "big")
            emit_proj_qk(0, 1, 0, 2, "big")
            for i in range(4):
                emit_proj_v(0, i, 2, "big")

            # qs0 fill: each unit lands before its first consumer (vS stile
            # i -> PV at step i//2; kT unit j -> QK(4j) at step 2j-1)
            fill = [qk_u(0, 1, 1), v_u(0, 4), v_u(0, 5),
                    qk_u(0, 1, 2), v_u(0, 6), v_u(0, 7),
                    v_u(0, 8), v_u(0, 9), qk_u(0, 1, 3),
                    v_u(0, 10), v_u(0, 11), v_u(0, 12),
                    v_u(0, 13), v_u(0, 14), v_u(0, 15),
                    qk_u(0, 0, 1)]
            emit_attention_qs(0, 0, fill)
            fill = [qk_u(0, 0, 2), qk_u(0, 0, 3)]
            emit_attention_qs(0, 1, fill)
            fill = []
            for t_idx in range(2):
                for j in range(4):
                    if t_idx == 0 and j >= 2:
                        continue  # deferred into attention(b1) slack
                    fill.append(qk_u(1, t_idx, j))
            for i in range(16):
                fill.append(v_u(1, i))
            emit_attention_qs(0, 2, fill)
            emit_attention_qs(0, 3, fill)
            while fill:
                fill.pop(0)()
            emit_recip_ship(0)
            emit_collective(0)

            # b1 attention with deferred b1 q-proj; b0 output interleaved
            # between q-slices (sits in engine FIFOs at natural slack points)
            ou = emit_output_units(0, out1, res_sb[0:2], 1, "aux")
            fill_b1 = [qk_u(1, 0, 2), qk_u(1, 0, 3)]
            emit_attention_qs(1, 0, fill_b1)
            emit_attention_qs(1, 1, fill_b1)
            ou[0]()  # bulk a2a_out load (collective 0 done by now)
            ou[1]()
            ou[2]()
            emit_attention_qs(1, 2, fill_b1)
            ou[3]()
            ou[4]()
            ou[5]()
            emit_attention_qs(1, 3, fill_b1)
            ou[6]()
            ou[7]()
            ou[8]()
            emit_recip_ship(1)
            emit_collective(1)
            for u in emit_output_units(1, out2, res_sb[2:4], 2, "big"):
                u()
    nc.finalize()
    return nc


def _prep_inputs(hidden_states, Wq, bq, Wk, bk, Wv, bv, Wo, bo):
    bf16 = ml_dtypes.bfloat16
    hs = np.asarray(hidden_states, np.float32)
    hsT = np.ascontiguousarray(
        hs.transpose(2, 0, 1).reshape(C, BS)).astype(bf16)
    Wo_h = np.ascontiguousarray(np.asarray(Wo, np.float32)).astype(bf16)
    in_maps = []
    for c in range(N_CORES):
        h0 = 2 * c
        cols = slice(64 * h0, 64 * h0 + 128)
        wv_c = np.zeros((C, 130), np.float32)
        bvb_c = np.zeros((1, 130), np.float32)
        for a in range(2):
            hd = slice(64 * (h0 + a), 64 * (h0 + a + 1))
            wv_c[:, 65 * a:65 * a + 64] = np.asarray(Wv, np.float32)[:, hd]
            bvb_c[0, 65 * a:65 * a + 64] = np.asarray(bv, np.float32)[hd]
            bvb_c[0, 65 * a + 64] = 1.0
        bqk_c = np.stack([np.asarray(bq, np.float32)[cols],
                          np.asarray(bk, np.float32)[cols]], axis=1)
        s0 = 256 * c
        bo_f = np.asarray(bo, np.float32)
        res_c = np.concatenate(
            [hs[0, s0:s0 + 256, :] + bo_f, hs[1, s0:s0 + 256, :] + bo_f],
            axis=0).astype(np.float32)
        in_maps.append({
            "hsT": hsT,
            "wq": np.ascontiguousarray(
                np.asarray(Wq, np.float32)[:, cols]).astype(bf16),
            "wk": np.ascontiguousarray(
                np.asarray(Wk, np.float32)[:, cols]).astype(bf16),
            "wv": wv_c.astype(bf16),
            "wo": Wo_h,
            "bqk": np.ascontiguousarray(bqk_c),
            "bvb": bvb_c,
            "res": np.ascontiguousarray(res_c),
            "esel": np.kron(np.eye(2, dtype=np.float32),
                            np.ones((1, 64), np.float32)).astype(bf16),
        })
    return in_maps


def _run(inputs, trace=False, trace_kwargs=None):
    if "nc" not in _CACHE:
        _CACHE["nc"] = _build()
    nc = _CACHE["nc"]
    in_maps = _prep_inputs(**inputs)
    r = run_bass_kernel_spmd(nc, in_maps, core_ids=list(range(N_CORES)),
                             trace=trace, **(trace_kwargs or {}))
    full = np.empty((B, S, C), np.float32)
    for c in range(N_CORES):
        full[0, 256 * c:256 * (c + 1), :] = r.results[c]["out1"]
        full[1, 256 * c:256 * (c + 1), :] = r.results[c]["out2"]
    return full, r


def kernel(**inputs):
    full, _ = _run(inputs, trace=False)
    return full


# revision 13
# speedup vs baseline: 1.1813x; 1.0032x over previous
"""Trainium2 Bass kernel for AttnProcessor self-attention (B=2,S=2048,C=1024,H=16).

Sharding: 8 cores, core c owns heads (2c, 2c+1) for both batches (tensor
parallel on the head dim for QKV); attention outputs are redistributed with
two 8-core AllToAlls (one per batch; both hidden under compute) so core c
computes the output projection + residual for output rows
(b, s in [256*c, 256*(c+1))). Host picks out1/out2 per core.

v2: all-bf16 compute (inputs/weights/probs/payload), f32 PSUM accumulate.
ScalarE runs exp exclusively; every DMA issues from sync/vector/gpsimd/
tensor queues. Both batches' hidden states + all weights stream in at t=0.
Output projection for batch 0 is interleaved into batch-1 attention as fill
work; softmax reciprocals ride the AllToAll (rows 128-129) and are
broadcast on the receiver with a tiny selector matmul on the idle PE.
"""
import numpy as np
import ml_dtypes

import concourse.bacc as bacc
import concourse.bass as bass
import concourse.tile as tile
from concourse import mybir
from concourse.bass_utils import run_bass_kernel_spmd

F32 = mybir.dt.float32
BF16 = mybir.dt.bfloat16

B, S, C, H, D = 2, 2048, 1024, 16, 64
N_CORES = 8
BS = B * S  # 4096
SCALE = 1.0 / np.sqrt(D)
SLAB = 130 * 256  # a2a slab stride

_CACHE = {}


def _build():
    nc = bacc.Bacc(num_devices=N_CORES)
    hsT = nc.declare_dram_parameter("hsT", [C, BS], BF16, isOutput=False)
    wq = nc.declare_dram_parameter("wq", [128, 1024], BF16, isOutput=False)
    wk = nc.declare_dram_parameter("wk", [128, 1024], BF16, isOutput=False)
    wv = nc.declare_dram_parameter("wv", [128, 1040], BF16, isOutput=False)
    wo = nc.declare_dram_parameter("wo", [C, C], BF16, isOutput=False)
    bqk = nc.declare_dram_parameter("bqk", [128, 2], F32, isOutput=False)
    bvb = nc.declare_dram_parameter("bvb", [1, 130], F32, isOutput=False)
    res = nc.declare_dram_parameter("res", [512, C], F32, isOutput=False)
    eselp = nc.declare_dram_parameter("esel", [2, 128], BF16, isOutput=False)
    out1 = nc.declare_dram_parameter("out1", [256, C], F32, isOutput=True)
    out2 = nc.declare_dram_parameter("out2", [256, C], F32, isOutput=True)

    with tile.TileContext(nc) as tc:
        with (
            tc.tile_pool(name="wpool", bufs=1) as wpool,
            tc.tile_pool(name="hpool", bufs=1) as hpool,
            tc.tile_pool(name="qkpool", bufs=2) as qkpool,
            tc.tile_pool(name="ppool", bufs=4) as ppool,
            tc.tile_pool(name="spool", bufs=3) as spool,
            tc.tile_pool(name="opool", bufs=2) as opool,
            tc.tile_pool(name="psum", bufs=1, space="PSUM") as psum,
            tc.tile_pool(name="dram", bufs=1, space="DRAM") as dram,
        ):
            rr = [nc.sync, nc.gpsimd]

            # ---- weight loads: host pre-arranged, contiguous ----
            def load_w(name, src, ncols, eng):
                t = wpool.tile([128, 8 * ncols], BF16, tag=name)
                eng.dma_start(out=t[:], in_=src[:])
                return [t[:, ncols * cc:ncols * (cc + 1)] for cc in range(8)]

            # scalar queue: ACT table warm-up, then hs b1 loads (all issue
            # long before the first exp needs the queue)
            warm = wpool.tile([1, 8], F32, tag="warm")
            nc.scalar.activation(warm[:], warm[:],
                                 mybir.ActivationFunctionType.Exp)
            wq_sb = load_w("wq", wq, 128, nc.sync)
            wk_sb = load_w("wk", wk, 128, nc.gpsimd)

            # hs tiles, both batches resident; b0 halves h=0 first
            hs = [[], []]
            for b in range(2):
                for cc in range(8):
                    t = hpool.tile([128, 2048], BF16, tag=f"hs{b}_{cc}",
                                   name=f"hs{b}_{cc}")
                    hs[b].append(t)
            for cc in range(8):
                rr[cc % 2].dma_start(
                    out=hs[0][cc][:, 0:1024],
                    in_=hsT[128 * cc:128 * (cc + 1), 0:1024])
            wv_sb = load_w("wv", wv, 130, nc.sync)  # [128,1040]
            bqk_sb = wpool.tile([128, 2], F32, tag="bqk")
            nc.gpsimd.dma_start(out=bqk_sb[:], in_=bqk[:])
            bvb_sb = wpool.tile([128, 130], F32, tag="bvb")
            bvb_ap = bvb[:]
            nc.gpsimd.dma_start(
                out=bvb_sb[:],
                in_=bass.AP(tensor=bvb_ap.tensor, offset=bvb_ap.offset,
                            ap=[[0, 128], [1, 130]]))
            for cc in range(8):
                rr[cc % 2].dma_start(
                    out=hs[0][cc][:, 1024:2048],
                    in_=hsT[128 * cc:128 * (cc + 1), 1024:2048])
            for cc in range(8):
                nc.scalar.dma_start(
                    out=hs[1][cc][:],
                    in_=hsT[128 * cc:128 * (cc + 1), 2048:4096])
            wo_sb = []
            for cc in range(8):
                t = wpool.tile([128, 1024], BF16, tag=f"wo{cc}",
                               name=f"wo{cc}")
                rr[cc % 2].dma_start(out=t[:],
                                     in_=wo[128 * cc:128 * (cc + 1), :])
                wo_sb.append(t)
            res_sb = []
            for st_i in range(4):
                t = wpool.tile([128, 1024], F32, tag=f"res{st_i}",
                               name=f"res{st_i}")
                rr[st_i % 2].dma_start(
                    out=t[:], in_=res[128 * st_i:128 * (st_i + 1), :])
                res_sb.append(t)
            # head-selector for reciprocal broadcast: esel[h, m] = (m//64==h)
            esel = wpool.tile([2, 128], BF16, tag="esel")
            nc.gpsimd.dma_start(out=esel[:], in_=eselp[:])

            a2a_in = [dram.tile([8, 130, 256], BF16, name=f"a2ain{b}")
                      for b in range(2)]
            a2a_out = [dram.tile([8, 130, 256], BF16, name=f"a2aout{b}")
                       for b in range(2)]

            qT, kT, vS, sums_pre = {}, {}, {}, {}

            def emit_proj_qk(b, t_idx, j, pool, tag):
                """One unit: tensor t_idx (0=q,1=k), one 512-wide s-slice j."""
                if t_idx == 0:
                    if b not in qT:
                        qT[b] = qkpool.tile([128, 2048], BF16, tag="qT",
                                            name=f"qT{b}")
                    dst, w_sb = qT[b], wq_sb
                else:
                    if b not in kT:
                        kT[b] = qkpool.tile([128, 2048], BF16, tag="kT",
                                            name=f"kT{b}")
                    dst, w_sb = kT[b], wk_sb
                ps = psum.tile([128, 1024], F32, tag=tag, bufs=pool,
                               name=f"pqk{b}_{t_idx}_{j}")
                sl = ps[:, 0:512]
                for cc in range(8):
                    nc.tensor.matmul(
                        sl, w_sb[cc],
                        hs[b][cc][:, 512 * j:512 * (j + 1)],
                        start=(cc == 0), stop=(cc == 7))
                nc.vector.tensor_scalar_add(
                    out=dst[:, 512 * j:512 * (j + 1)], in0=sl,
                    scalar1=bqk_sb[:, t_idx:t_idx + 1])

            def emit_proj_v(b, i, pool, tag):
                """One unit: one 128-row v' s-tile i."""
                if b not in vS:
                    vS[b] = qkpool.tile([128, 2080], BF16, tag="vS",
                                        name=f"vS{b}")
                dst = vS[b]
                ps = psum.tile([128, 1024], F32, tag=tag, bufs=pool,
                               name=f"pv{b}_{i}")
                sl = ps[:, 0:130]
                for cc in range(8):
                    nc.tensor.matmul(
                        sl, hs[b][cc][:, 128 * i:128 * (i + 1)], wv_sb[cc],
                        start=(cc == 0), stop=(cc == 7))
                nc.vector.tensor_tensor(
                    out=dst[:, 130 * i:130 * (i + 1)], in0=sl,
                    in1=bvb_sb[:, 0:130], op=mybir.AluOpType.add)

            def emit_attention_qs(b, qs, fill_work):
                """One q-slice (512 q) for both heads, kc-pair steps."""
                accA = psum.tile([65, 512], F32, tag="accA", bufs=1,
                                 name=f"accA_{b}_{qs}")
                accB = psum.tile([65, 512], F32, tag="accB", bufs=1,
                                 name=f"accB_{b}_{qs}")
                sc_t = {}

                def emit_qk(kc):
                    sc = psum.tile([128, 1024], F32, tag="big", bufs=2,
                                   name=f"sc_{b}_{qs}_{kc}")
                    sc_t[kc] = sc
                    nc.tensor.matmul(
                        sc[:, 0:512],
                        kT[b][0:64, 128 * kc:128 * (kc + 1)],
                        qT[b][0:64, 512 * qs:512 * (qs + 1)],
                        start=True, stop=True)
                    nc.tensor.matmul(
                        sc[:, 512:1024],
                        kT[b][64:128, 128 * kc:128 * (kc + 1)],
                        qT[b][64:128, 512 * qs:512 * (qs + 1)],
                        start=True, stop=True)

                def emit_pv(acc, off, kc, pr):
                    nc.tensor.matmul(
                        acc[:],
                        vS[b][:, 130 * kc + off:130 * kc + off + 65],
                        pr[:, (0 if off == 0 else 512):
                           (512 if off == 0 else 1024)],
                        start=(kc == 0), stop=(kc == 15))

                emit_qk(0)
                emit_qk(1)
                for step in range(8):
                    kc0, kc1 = 2 * step, 2 * step + 1
                    for _ in range(2):
                        if fill_work:
                            fill_work.pop(0)()
                    pr0 = ppool.tile([128, 1024], BF16, tag="pr", bufs=4,
                                     name=f"pr_{b}_{qs}_{kc0}")
                    nc.scalar.activation(pr0[:], sc_t.pop(kc0)[:],
                                         mybir.ActivationFunctionType.Exp,
                                         scale=float(SCALE))
                    pr1 = ppool.tile([128, 1024], BF16, tag="pr", bufs=4,
                                     name=f"pr_{b}_{qs}_{kc1}")
                    nc.scalar.activation(pr1[:], sc_t.pop(kc1)[:],
                                         mybir.ActivationFunctionType.Exp,
                                         scale=float(SCALE))
                    if step < 7:
                        emit_qk(kc0 + 2)
                        emit_qk(kc1 + 2)
                    emit_pv(accA, 0, kc0, pr0)
                    emit_pv(accA, 0, kc1, pr1)
                    emit_pv(accB, 65, kc0, pr0)
                    emit_pv(accB, 65, kc1, pr1)
                # drain: rows [0:64] -> a2a_in slabs, row 64 -> sums_pre
                if b not in sums_pre:
                    sums_pre[b] = spool.tile([128, 32], BF16, tag="sums",
                                             name=f"sums{b}")
                for h, acc in ((0, accA), (1, accB)):
                    st = spool.tile([65, 512], BF16, tag="st",
                                    name=f"st_{b}_{qs}_{h}")
                    nc.vector.tensor_copy(st[:], acc[:])
                    dst = a2a_in[b][:]
                    nc.sync.dma_start(
                        out=bass.AP(
                            tensor=dst.tensor,
                            offset=(dst.offset + (2 * qs) * SLAB
                                    + 64 * h * 256),
                            ap=[[256, 64], [SLAB, 2], [1, 256]]),
                        in_=st[0:64, :])
                    r = 2 * qs + h
                    nc.sync.dma_start(
                        out=sums_pre[b][16 * r:16 * (r + 1), :],
                        in_=st[64:65, :])

            def emit_recip_ship(b):
                with nc.allow_low_precision("bf16 softmax denominators"):
                    nc.vector.reciprocal(sums_pre[b][:], sums_pre[b][:])
                for j in range(8):
                    qs, half = j // 2, j % 2
                    dst = a2a_in[b][:]
                    for h in range(2):
                        r = 2 * qs + h
                        nc.sync.dma_start(
                            out=bass.AP(
                                tensor=dst.tensor,
                                offset=(dst.offset + j * SLAB
                                        + (128 + h) * 256),
                                ap=[[1, 256]]),
                            in_=sums_pre[b][16 * r + 8 * half:
                                            16 * r + 8 * (half + 1), :])

            def emit_collective(b):
                nc.gpsimd.collective_compute(
                    "AllToAll", mybir.AluOpType.bypass,
                    replica_groups=[list(range(8))],
                    ins=[a2a_in[b][:]], outs=[a2a_out[b][:]])

            def emit_output_units(b, out_t, res_pair, bc_pt, op_pt):
                """Returns list of units: bulk load, recip bcast+normalize,
                out-projection (2 q-halves), residual+store."""
                state = {}

                def u_load():
                    raw = opool.tile([128, 2048], BF16, tag="raw",
                                     name=f"raw{b}")
                    rap = a2a_out[b][:]
                    nc.sync.dma_start(
                        out=raw[:],
                        in_=bass.AP(tensor=rap.tensor, offset=rap.offset,
                                    ap=[[256, 128], [SLAB, 8], [1, 256]]))
                    rc = opool.tile([2, 2048], BF16, tag="rcp",
                                    name=f"rcp{b}")
                    nc.sync.dma_start(
                        out=rc[:],
                        in_=bass.AP(tensor=rap.tensor,
                                    offset=rap.offset + 128 * 256,
                                    ap=[[256, 2], [SLAB, 8], [1, 256]]))
                    state["raw"], state["rc"] = raw, rc
                    state["an"] = opool.tile([128, 2048], BF16, tag="an",
                                             name=f"an{b}")

                def u_norm(g):
                    def f():
                        bc = psum.tile([128, 1024], F32, tag=bc_pt[1],
                                       bufs=bc_pt[0], name=f"bc{b}_{g}")
                        for hf in range(2):
                            nc.tensor.matmul(
                                bc[:, 512 * hf:512 * (hf + 1)], esel[:],
                                state["rc"][:, 1024 * g + 512 * hf:
                                            1024 * g + 512 * (hf + 1)],
                                start=True, stop=True)
                        nc.vector.tensor_tensor(
                            out=state["an"][:, 1024 * g:1024 * (g + 1)],
                            in0=state["raw"][:, 1024 * g:1024 * (g + 1)],
                            in1=bc[:], op=mybir.AluOpType.mult)
                    return f

                def u_proj(st_i, jlo, jhi, first):
                    def f():
                        if first:
                            state[f"op{st_i}"] = psum.tile(
                                [128, 1024], F32, tag=op_pt[1],
                                bufs=op_pt[0], name=f"op{b}_{st_i}")
                        ps = state[f"op{st_i}"]
                        an = state["an"]
                        for j in range(jlo, jhi):
                            for co in range(2):
                                nc.tensor.matmul(
                                    ps[:, 512 * co:512 * (co + 1)],
                                    an[:, 256 * j + 128 * st_i:
                                       256 * j + 128 * st_i + 128],
                                    wo_sb[j][:, 512 * co:512 * (co + 1)],
                                    start=(j == 0), stop=(j == 7))
                    return f

                def u_store(st_i):
                    def f():
                        ob = opool.tile([128, 1024], F32, tag="ob",
                                        name=f"ob{b}_{st_i}")
                        nc.vector.tensor_tensor(
                            out=ob[:], in0=state[f"op{st_i}"][:],
                            in1=res_pair[st_i][:], op=mybir.AluOpType.add)
                        nc.sync.dma_start(
                            out=out_t[128 * st_i:128 * (st_i + 1), :],
                            in_=ob[:])
                    return f

                return [u_load, u_norm(0), u_norm(1),
                        u_proj(0, 0, 4, True), u_proj(0, 4, 8, False),
                        u_store(0),
                        u_proj(1, 0, 4, True), u_proj(1, 4, 8, False),
                        u_store(1)]

            # ---------------- emission ----------------
            def qk_u(b, t, j, pool=1, tag="aux"):
                return lambda: emit_proj_qk(b, t, j, pool, tag)

            def v_u(b, i, pool=1, tag="aux"):
                return lambda: emit_proj_v(b, i, pool, tag)

            # prefix: just enough b0 projection for attention(b0, qs0) kc 0-3
            emit_proj_qk(0, 0, 0, 2, "big")
            emit_proj_qk(0, 1, 0, 2, "big")
            for i in range(4):
                emit_proj_v(0, i, 2, "big")

            # qs0 fill: each unit lands before its first consumer (vS stile
            # i -> PV at step i//2; kT unit j -> QK(4j) at step 2j-1)
            fill = [qk_u(0, 1, 1), v_u(0, 4), v_u(0, 5),
                    qk_u(0, 1, 2), v_u(0, 6), v_u(0, 7),
                    v_u(0, 8), v_u(0, 9), qk_u(0, 1, 3),
                    v_u(0, 10), v_u(0, 11), v_u(0, 12),
                    v_u(0, 13), v_u(0, 14), v_u(0, 15),
                    qk_u(0, 0, 1)]
            emit_attention_qs(0, 0, fill)
            fill = [qk_u(0, 0, 2), qk_u(0, 0, 3)]
            emit_attention_qs(0, 1, fill)
            fill = []
            for t_idx in range(2):
                for j in range(4):
                    if t_idx == 0 and j >= 2:
                        continue  # deferred into attention(b1) slack
                    fill.append(qk_u(1, t_idx, j))
            for i in range(16):
                fill.append(v_u(1, i))
            emit_attention_qs(0, 2, fill)
            emit_attention_qs(0, 3, fill)
            while fill:
                fill.pop(0)()
            emit_recip_ship(0)
            emit_collective(0)

            # b1 attention with deferred b1 q-proj; b0 output interleaved
            # between q-slices (sits in engine FIFOs at natural slack points)
            ou = emit_output_units(0, out1, res_sb[0:2],
                                   (1, "aux"), (1, "aux"))
            fill_b1 = [qk_u(1, 0, 2), qk_u(1, 0, 3)]
            emit_attention_qs(1, 0, fill_b1)
            emit_attention_qs(1, 1, fill_b1)
            ou[0]()  # bulk a2a_out load (collective 0 done by now)
            ou[1]()
            ou[2]()
            emit_attention_qs(1, 2, fill_b1)
            ou[3]()
            ou[4]()
            ou[5]()
            emit_attention_qs(1, 3, fill_b1)
            ou[6]()
            ou[7]()
            ou[8]()
            emit_recip_ship(1)
            emit_collective(1)
            ou1 = emit_output_units(1, out2, res_sb[2:4],
                                    (1, "aux"), (2, "big"))
            for idx in (0, 1, 3, 2, 4, 6, 5, 7, 8):
                ou1[idx]()
    nc.finalize()
    return nc


def _prep_inputs(hidden_states, Wq, bq, Wk, bk, Wv, bv, Wo, bo):
    bf16 = ml_dtypes.bfloat16
    hs = np.asarray(hidden_states, np.float32)
    hsT = np.ascontiguousarray(
        hs.transpose(2, 0, 1).reshape(C, BS)).astype(bf16)
    Wo_h = np.ascontiguousarray(np.asarray(Wo, np.float32)).astype(bf16)
    in_maps = []
    def prearrange(w):
        # [C, ncols] -> [128, 8*ncols]: sbuf partition p, col ncols*cc+m
        ncols = w.shape[1]
        return np.ascontiguousarray(
            w.reshape(8, 128, ncols).transpose(1, 0, 2).reshape(
                128, 8 * ncols))

    for c in range(N_CORES):
        h0 = 2 * c
        cols = slice(64 * h0, 64 * h0 + 128)
        wv_c = np.zeros((C, 130), np.float32)
        bvb_c = np.zeros((1, 130), np.float32)
        for a in range(2):
            hd = slice(64 * (h0 + a), 64 * (h0 + a + 1))
            wv_c[:, 65 * a:65 * a + 64] = np.asarray(Wv, np.float32)[:, hd]
            bvb_c[0, 65 * a:65 * a + 64] = np.asarray(bv, np.float32)[hd]
            bvb_c[0, 65 * a + 64] = 1.0
        bqk_c = np.stack([np.asarray(bq, np.float32)[cols],
                          np.asarray(bk, np.float32)[cols]], axis=1)
        s0 = 256 * c
        bo_f = np.asarray(bo, np.float32)
        res_c = np.concatenate(
            [hs[0, s0:s0 + 256, :] + bo_f, hs[1, s0:s0 + 256, :] + bo_f],
            axis=0).astype(np.float32)
        in_maps.append({
            "hsT": hsT,
            "wq": prearrange(
                np.asarray(Wq, np.float32)[:, cols]).astype(bf16),
            "wk": prearrange(
                np.asarray(Wk, np.float32)[:, cols]).astype(bf16),
            "wv": prearrange(wv_c).astype(bf16),
            "wo": Wo_h,
            "bqk": np.ascontiguousarray(bqk_c),
            "bvb": bvb_c,
            "res": np.ascontiguousarray(res_c),
            "esel": np.kron(np.eye(2, dtype=np.float32),
                            np.ones((1, 64), np.float32)).astype(bf16),
        })
    return in_maps


def _run(inputs, trace=False, trace_kwargs=None):
    if "nc" not in _CACHE:
        _CACHE["nc"] = _build()
    nc = _CACHE["nc"]
    in_maps = _prep_inputs(**inputs)
    r = run_bass_kernel_spmd(nc, in_maps, core_ids=list(range(N_CORES)),
                             trace=trace, **(trace_kwargs or {}))
    full = np.empty((B, S, C), np.float32)
    for c in range(N_CORES):
        full[0, 256 * c:256 * (c + 1), :] = r.results[c]["out1"]
        full[1, 256 * c:256 * (c + 1), :] = r.results[c]["out2"]
    return full, r


def kernel(**inputs):
    full, _ = _run(inputs, trace=False)
    return full


# revision 14
# speedup vs baseline: 1.3036x; 1.1035x over previous
"""Trainium2 Bass kernel for AttnProcessor self-attention (B=2,S=2048,C=1024,H=16).

Sharding: 8 cores, core c owns heads (2c, 2c+1) for both batches (tensor
parallel on the head dim for QKV); attention outputs are redistributed with
two 8-core AllToAlls (one per batch; both hidden under compute) so core c
computes the output projection + residual for output rows
(b, s in [256*c, 256*(c+1))). Host picks out1/out2 per core.

v2: all-bf16 compute (inputs/weights/probs/payload), f32 PSUM accumulate.
ScalarE runs exp exclusively; every DMA issues from sync/vector/gpsimd/
tensor queues. Both batches' hidden states + all weights stream in at t=0.
Output projection for batch 0 is interleaved into batch-1 attention as fill
work; softmax reciprocals ride the AllToAll (rows 128-129) and are
broadcast on the receiver with a tiny selector matmul on the idle PE.
"""
import numpy as np
import ml_dtypes

import concourse.bacc as bacc
import concourse.bass as bass
import concourse.tile as tile
from concourse import mybir
from concourse.bass_utils import run_bass_kernel_spmd

F32 = mybir.dt.float32
BF16 = mybir.dt.bfloat16

B, S, C, H, D = 2, 2048, 1024, 16, 64
N_CORES = 8
BS = B * S  # 4096
SCALE = 1.0 / np.sqrt(D)
SLAB = 130 * 256  # a2a slab stride

_CACHE = {}


def _build():
    nc = bacc.Bacc(num_devices=N_CORES)
    hsT = nc.declare_dram_parameter("hsT", [C, BS], BF16, isOutput=False)
    wq = nc.declare_dram_parameter("wq", [128, 1024], BF16, isOutput=False)
    wk = nc.declare_dram_parameter("wk", [128, 1024], BF16, isOutput=False)
    wv = nc.declare_dram_parameter("wv", [128, 1040], BF16, isOutput=False)
    wo = nc.declare_dram_parameter("wo", [C, C], BF16, isOutput=False)
    bqk = nc.declare_dram_parameter("bqk", [128, 2], F32, isOutput=False)
    bvb = nc.declare_dram_parameter("bvb", [1, 130], F32, isOutput=False)
    res = nc.declare_dram_parameter("res", [512, C], F32, isOutput=False)
    eselp = nc.declare_dram_parameter("esel", [2, 128], BF16, isOutput=False)
    out1 = nc.declare_dram_parameter("out1", [256, C], F32, isOutput=True)
    out2 = nc.declare_dram_parameter("out2", [256, C], F32, isOutput=True)

    with tile.TileContext(nc) as tc:
        with (
            tc.tile_pool(name="wpool", bufs=1) as wpool,
            tc.tile_pool(name="hpool", bufs=1) as hpool,
            tc.tile_pool(name="qkpool", bufs=2) as qkpool,
            tc.tile_pool(name="ppool", bufs=6) as ppool,
            tc.tile_pool(name="spool", bufs=4) as spool,
            tc.tile_pool(name="opool", bufs=2) as opool,
            tc.tile_pool(name="psum", bufs=1, space="PSUM") as psum,
            tc.tile_pool(name="dram", bufs=1, space="DRAM") as dram,
        ):
            rr = [nc.sync, nc.gpsimd]

            # ---- weight loads: host pre-arranged, contiguous ----
            def load_w(name, src, ncols, eng):
                t = wpool.tile([128, 8 * ncols], BF16, tag=name)
                eng.dma_start(out=t[:], in_=src[:])
                return [t[:, ncols * cc:ncols * (cc + 1)] for cc in range(8)]

            # scalar queue: ACT table warm-up, then hs b1 loads (all issue
            # long before the first exp needs the queue)
            warm = wpool.tile([1, 8], F32, tag="warm")
            nc.scalar.activation(warm[:], warm[:],
                                 mybir.ActivationFunctionType.Exp)
            wq_sb = load_w("wq", wq, 128, nc.sync)
            wk_sb = load_w("wk", wk, 128, nc.gpsimd)

            # hs tiles, both batches resident; full batch-rows = 4KB
            # contiguous lines per DMA; spread over sync/gpsimd/scalar
            rr3 = [nc.sync, nc.gpsimd, nc.scalar]
            hs = [[], []]
            for b in range(2):
                for cc in range(8):
                    t = hpool.tile([128, 2048], BF16, tag=f"hs{b}_{cc}",
                                   name=f"hs{b}_{cc}")
                    hs[b].append(t)
            for cc in range(8):
                rr3[cc % 3].dma_start(
                    out=hs[0][cc][:],
                    in_=hsT[128 * cc:128 * (cc + 1), 0:2048])
            wv_sb = load_w("wv", wv, 130, nc.sync)  # [128,1040]
            bqk_sb = wpool.tile([128, 2], F32, tag="bqk")
            nc.gpsimd.dma_start(out=bqk_sb[:], in_=bqk[:])
            bvb_sb = wpool.tile([128, 130], F32, tag="bvb")
            bvb_ap = bvb[:]
            nc.gpsimd.dma_start(
                out=bvb_sb[:],
                in_=bass.AP(tensor=bvb_ap.tensor, offset=bvb_ap.offset,
                            ap=[[0, 128], [1, 130]]))
            for cc in range(8):
                rr3[(cc + 2) % 3].dma_start(
                    out=hs[1][cc][:],
                    in_=hsT[128 * cc:128 * (cc + 1), 2048:4096])
            wo_sb = []
            for cc in range(8):
                t = wpool.tile([128, 1024], BF16, tag=f"wo{cc}",
                               name=f"wo{cc}")
                rr[cc % 2].dma_start(out=t[:],
                                     in_=wo[128 * cc:128 * (cc + 1), :])
                wo_sb.append(t)
            res_sb = []
            for st_i in range(4):
                t = wpool.tile([128, 1024], F32, tag=f"res{st_i}",
                               name=f"res{st_i}")
                rr[st_i % 2].dma_start(
                    out=t[:], in_=res[128 * st_i:128 * (st_i + 1), :])
                res_sb.append(t)
            # head-selector for reciprocal broadcast: esel[h, m] = (m//64==h)
            esel = wpool.tile([2, 128], BF16, tag="esel")
            nc.gpsimd.dma_start(out=esel[:], in_=eselp[:])

            a2a_in = [dram.tile([8, 130, 256], BF16, name=f"a2ain{b}")
                      for b in range(2)]
            a2a_out = [dram.tile([8, 130, 256], BF16, name=f"a2aout{b}")
                       for b in range(2)]
            # dummy collective: absorbs CC-stream warm-up + the inter-core
            # rendezvous skew, off the critical path
            cwarm_i = dram.tile([8, 1, 16], BF16, name="cwarm_i")
            cwarm_o = dram.tile([8, 1, 16], BF16, name="cwarm_o")
            nc.gpsimd.collective_compute(
                "AllToAll", mybir.AluOpType.bypass,
                replica_groups=[list(range(8))],
                ins=[cwarm_i[:]], outs=[cwarm_o[:]])

            qT, kT, vS, sums_pre = {}, {}, {}, {}

            def emit_proj_qk(b, t_idx, j, pool, tag):
                """One unit: tensor t_idx (0=q,1=k), one 512-wide s-slice j."""
                if t_idx == 0:
                    if b not in qT:
                        qT[b] = qkpool.tile([128, 2048], BF16, tag="qT",
                                            name=f"qT{b}")
                    dst, w_sb = qT[b], wq_sb
                else:
                    if b not in kT:
                        kT[b] = qkpool.tile([128, 2048], BF16, tag="kT",
                                            name=f"kT{b}")
                    dst, w_sb = kT[b], wk_sb
                ps = psum.tile([128, 1024], F32, tag=tag, bufs=pool,
                               name=f"pqk{b}_{t_idx}_{j}")
                sl = ps[:, 0:512]
                for cc in range(8):
                    nc.tensor.matmul(
                        sl, w_sb[cc],
                        hs[b][cc][:, 512 * j:512 * (j + 1)],
                        start=(cc == 0), stop=(cc == 7))
                nc.vector.tensor_scalar_add(
                    out=dst[:, 512 * j:512 * (j + 1)], in0=sl,
                    scalar1=bqk_sb[:, t_idx:t_idx + 1])

            def emit_proj_v(b, i, pool, tag):
                """One unit: one 128-row v' s-tile i."""
                if b not in vS:
                    vS[b] = qkpool.tile([128, 2080], BF16, tag="vS",
                                        name=f"vS{b}")
                dst = vS[b]
                ps = psum.tile([128, 1024], F32, tag=tag, bufs=pool,
                               name=f"pv{b}_{i}")
                sl = ps[:, 0:130]
                for cc in range(8):
                    nc.tensor.matmul(
                        sl, hs[b][cc][:, 128 * i:128 * (i + 1)], wv_sb[cc],
                        start=(cc == 0), stop=(cc == 7))
                nc.vector.tensor_tensor(
                    out=dst[:, 130 * i:130 * (i + 1)], in0=sl,
                    in1=bvb_sb[:, 0:130], op=mybir.AluOpType.add)

            def emit_attention_qs(b, qs, fill_work):
                """One q-slice (512 q) for both heads, kc-pair steps."""
                accA = psum.tile([65, 512], F32, tag="accA", bufs=1,
                                 name=f"accA_{b}_{qs}")
                accB = psum.tile([65, 512], F32, tag="accB", bufs=1,
                                 name=f"accB_{b}_{qs}")
                sc_t = {}

                def emit_qk(kc):
                    sc = psum.tile([128, 1024], F32, tag="big", bufs=2,
                                   name=f"sc_{b}_{qs}_{kc}")
                    sc_t[kc] = sc
                    nc.tensor.matmul(
                        sc[:, 0:512],
                        kT[b][0:64, 128 * kc:128 * (kc + 1)],
                        qT[b][0:64, 512 * qs:512 * (qs + 1)],
                        start=True, stop=True)
                    nc.tensor.matmul(
                        sc[:, 512:1024],
                        kT[b][64:128, 128 * kc:128 * (kc + 1)],
                        qT[b][64:128, 512 * qs:512 * (qs + 1)],
                        start=True, stop=True)

                def emit_pv(acc, off, kc, pr):
                    nc.tensor.matmul(
                        acc[:],
                        vS[b][:, 130 * kc + off:130 * kc + off + 65],
                        pr[:, (0 if off == 0 else 512):
                           (512 if off == 0 else 1024)],
                        start=(kc == 0), stop=(kc == 15))

                emit_qk(0)
                emit_qk(1)
                for step in range(8):
                    kc0, kc1 = 2 * step, 2 * step + 1
                    for _ in range(2):
                        if fill_work:
                            fill_work.pop(0)()
                    pr0 = ppool.tile([128, 1024], BF16, tag="pr", bufs=6,
                                     name=f"pr_{b}_{qs}_{kc0}")
                    nc.scalar.activation(pr0[:], sc_t.pop(kc0)[:],
                                         mybir.ActivationFunctionType.Exp,
                                         scale=float(SCALE))
                    pr1 = ppool.tile([128, 1024], BF16, tag="pr", bufs=6,
                                     name=f"pr_{b}_{qs}_{kc1}")
                    nc.scalar.activation(pr1[:], sc_t.pop(kc1)[:],
                                         mybir.ActivationFunctionType.Exp,
                                         scale=float(SCALE))
                    if step < 7:
                        emit_qk(kc0 + 2)
                        emit_qk(kc1 + 2)
                    emit_pv(accA, 0, kc0, pr0)
                    emit_pv(accA, 0, kc1, pr1)
                    emit_pv(accB, 65, kc0, pr0)
                    emit_pv(accB, 65, kc1, pr1)
                # drain: rows [0:64] -> a2a_in slabs, row 64 -> sums_pre
                if b not in sums_pre:
                    sums_pre[b] = spool.tile([128, 32], BF16, tag="sums",
                                             name=f"sums{b}")
                for h, acc in ((0, accA), (1, accB)):
                    st = spool.tile([65, 512], BF16, tag="st",
                                    name=f"st_{b}_{qs}_{h}")
                    nc.vector.tensor_copy(st[:], acc[:])
                    dst = a2a_in[b][:]
                    nc.sync.dma_start(
                        out=bass.AP(
                            tensor=dst.tensor,
                            offset=(dst.offset + (2 * qs) * SLAB
                                    + 64 * h * 256),
                            ap=[[256, 64], [SLAB, 2], [1, 256]]),
                        in_=st[0:64, :])
                    r = 2 * qs + h
                    nc.sync.dma_start(
                        out=sums_pre[b][16 * r:16 * (r + 1), :],
                        in_=st[64:65, :])
                with nc.allow_low_precision("bf16 softmax denominators"):
                    nc.vector.reciprocal(
                        sums_pre[b][32 * qs:32 * (qs + 1), :],
                        sums_pre[b][32 * qs:32 * (qs + 1), :])
                for j in (2 * qs, 2 * qs + 1):
                    half = j % 2
                    dst = a2a_in[b][:]
                    for h in range(2):
                        r = 2 * qs + h
                        nc.sync.dma_start(
                            out=bass.AP(
                                tensor=dst.tensor,
                                offset=(dst.offset + j * SLAB
                                        + (128 + h) * 256),
                                ap=[[1, 256]]),
                            in_=sums_pre[b][16 * r + 8 * half:
                                            16 * r + 8 * (half + 1), :])

            def emit_collective(b):
                nc.gpsimd.collective_compute(
                    "AllToAll", mybir.AluOpType.bypass,
                    replica_groups=[list(range(8))],
                    ins=[a2a_in[b][:]], outs=[a2a_out[b][:]])

            def emit_output_units(b, out_t, res_pair, bc_pt, op_pt):
                """Returns list of units: bulk load, recip bcast+normalize,
                out-projection (2 q-halves), residual+store."""
                state = {}

                def u_load():
                    raw = opool.tile([128, 2048], BF16, tag="raw",
                                     name=f"raw{b}")
                    rap = a2a_out[b][:]
                    nc.gpsimd.dma_start(
                        out=raw[:],
                        in_=bass.AP(tensor=rap.tensor, offset=rap.offset,
                                    ap=[[256, 128], [SLAB, 8], [1, 256]]))
                    rc = opool.tile([2, 2048], BF16, tag="rcp",
                                    name=f"rcp{b}")
                    nc.gpsimd.dma_start(
                        out=rc[:],
                        in_=bass.AP(tensor=rap.tensor,
                                    offset=rap.offset + 128 * 256,
                                    ap=[[256, 2], [SLAB, 8], [1, 256]]))
                    state["raw"], state["rc"] = raw, rc
                    state["an"] = opool.tile([128, 2048], BF16, tag="an",
                                             name=f"an{b}")

                def u_norm(g):
                    def f():
                        bc = psum.tile([128, 1024], F32, tag=bc_pt[1],
                                       bufs=bc_pt[0], name=f"bc{b}_{g}")
                        for hf in range(2):
                            nc.tensor.matmul(
                                bc[:, 512 * hf:512 * (hf + 1)], esel[:],
                                state["rc"][:, 1024 * g + 512 * hf:
                                            1024 * g + 512 * (hf + 1)],
                                start=True, stop=True)
                        nc.vector.tensor_tensor(
                            out=state["an"][:, 1024 * g:1024 * (g + 1)],
                            in0=state["raw"][:, 1024 * g:1024 * (g + 1)],
                            in1=bc[:], op=mybir.AluOpType.mult)
                    return f

                def u_proj(st_i, jlo, jhi, first):
                    def f():
                        if first:
                            state[f"op{st_i}"] = psum.tile(
                                [128, 1024], F32, tag=op_pt[1],
                                bufs=op_pt[0], name=f"op{b}_{st_i}")
                        ps = state[f"op{st_i}"]
                        an = state["an"]
                        for j in range(jlo, jhi):
                            for co in range(2):
                                nc.tensor.matmul(
                                    ps[:, 512 * co:512 * (co + 1)],
                                    an[:, 256 * j + 128 * st_i:
                                       256 * j + 128 * st_i + 128],
                                    wo_sb[j][:, 512 * co:512 * (co + 1)],
                                    start=(j == 0), stop=(j == 7))
                    return f

                def u_store(st_i):
                    def f():
                        ob = opool.tile([128, 1024], F32, tag="ob",
                                        name=f"ob{b}_{st_i}")
                        nc.vector.tensor_tensor(
                            out=ob[:], in0=state[f"op{st_i}"][:],
                            in1=res_pair[st_i][:], op=mybir.AluOpType.add)
                        nc.sync.dma_start(
                            out=out_t[128 * st_i:128 * (st_i + 1), :],
                            in_=ob[:])
                    return f

                return [u_load, u_norm(0), u_norm(1),
                        u_proj(0, 0, 4, True), u_proj(0, 4, 8, False),
                        u_store(0),
                        u_proj(1, 0, 4, True), u_proj(1, 4, 8, False),
                        u_store(1)]

            # ---------------- emission ----------------
            def qk_u(b, t, j, pool=1, tag="aux"):
                return lambda: emit_proj_qk(b, t, j, pool, tag)

            def v_u(b, i, pool=1, tag="aux"):
                return lambda: emit_proj_v(b, i, pool, tag)

            # prefix: just enough b0 projection for attention(b0, qs0) kc 0-3
            emit_proj_qk(0, 0, 0, 2, "big")
            emit_proj_qk(0, 1, 0, 2, "big")
            for i in range(4):
                emit_proj_v(0, i, 2, "big")

            # qs0 fill: each unit lands before its first consumer (vS stile
            # i -> PV at step i//2; kT unit j -> QK(4j) at step 2j-1)
            fill = [qk_u(0, 1, 1), v_u(0, 4), v_u(0, 5),
                    qk_u(0, 1, 2), v_u(0, 6), v_u(0, 7),
                    v_u(0, 8), v_u(0, 9), qk_u(0, 1, 3),
                    v_u(0, 10), v_u(0, 11), v_u(0, 12),
                    v_u(0, 13), v_u(0, 14), v_u(0, 15),
                    qk_u(0, 0, 1)]
            emit_attention_qs(0, 0, fill)
            fill = [qk_u(0, 0, 2), qk_u(0, 0, 3),
                    qk_u(1, 1, 0), qk_u(1, 1, 1)]
            emit_attention_qs(0, 1, fill)
            fill += [qk_u(1, 1, 2), qk_u(1, 1, 3),
                     qk_u(1, 0, 0), qk_u(1, 0, 1)]
            fill += [v_u(1, i) for i in range(4)]
            emit_attention_qs(0, 2, fill)
            fill += [v_u(1, i) for i in range(4, 16)]
            emit_attention_qs(0, 3, fill)
            while fill:
                fill.pop(0)()
            emit_collective(0)

            # b1 attention with deferred b1 q-proj; b0 output interleaved
            # between q-slices (sits in engine FIFOs at natural slack points)
            ou = emit_output_units(0, out1, res_sb[0:2],
                                   (1, "aux"), (1, "aux"))
            fill_b1 = [qk_u(1, 0, 2), qk_u(1, 0, 3)]
            emit_attention_qs(1, 0, fill_b1)
            emit_attention_qs(1, 1, fill_b1)
            ou[0]()  # bulk a2a_out load (collective 0 done by now)
            ou[1]()
            ou[2]()
            emit_attention_qs(1, 2, fill_b1)
            ou[3]()
            ou[4]()
            ou[5]()
            emit_attention_qs(1, 3, fill_b1)
            ou[6]()
            ou[7]()
            ou[8]()
            emit_collective(1)
            ou1 = emit_output_units(1, out2, res_sb[2:4],
                                    (1, "aux"), (2, "big"))
            for idx in (0, 1, 3, 2, 4, 6, 5, 7, 8):
                ou1[idx]()
    nc.finalize()
    return nc


def _prep_inputs(hidden_states, Wq, bq, Wk, bk, Wv, bv, Wo, bo):
    bf16 = ml_dtypes.bfloat16
    hs = np.asarray(hidden_states, np.float32)
    hsT = np.ascontiguousarray(
        hs.transpose(2, 0, 1).reshape(C, BS)).astype(bf16)
    Wo_h = np.ascontiguousarray(np.asarray(Wo, np.float32)).astype(bf16)
    in_maps = []
    def prearrange(w):
        # [C, ncols] -> [128, 8*ncols]: sbuf partition p, col ncols*cc+m
        ncols = w.shape[1]
        return np.ascontiguousarray(
            w.reshape(8, 128, ncols).transpose(1, 0, 2).reshape(
                128, 8 * ncols))

    for c in range(N_CORES):
        h0 = 2 * c
        cols = slice(64 * h0, 64 * h0 + 128)
        wv_c = np.zeros((C, 130), np.float32)
        bvb_c = np.zeros((1, 130), np.float32)
        for a in range(2):
            hd = slice(64 * (h0 + a), 64 * (h0 + a + 1))
            wv_c[:, 65 * a:65 * a + 64] = np.asarray(Wv, np.float32)[:, hd]
            bvb_c[0, 65 * a:65 * a + 64] = np.asarray(bv, np.float32)[hd]
            bvb_c[0, 65 * a + 64] = 1.0
        bqk_c = np.stack([np.asarray(bq, np.float32)[cols],
                          np.asarray(bk, np.float32)[cols]], axis=1)
        s0 = 256 * c
        bo_f = np.asarray(bo, np.float32)
        res_c = np.concatenate(
            [hs[0, s0:s0 + 256, :] + bo_f, hs[1, s0:s0 + 256, :] + bo_f],
            axis=0).astype(np.float32)
        in_maps.append({
            "hsT": hsT,
            "wq": prearrange(
                np.asarray(Wq, np.float32)[:, cols]).astype(bf16),
            "wk": prearrange(
                np.asarray(Wk, np.float32)[:, cols]).astype(bf16),
            "wv": prearrange(wv_c).astype(bf16),
            "wo": Wo_h,
            "bqk": np.ascontiguousarray(bqk_c),
            "bvb": bvb_c,
            "res": np.ascontiguousarray(res_c),
            "esel": np.kron(np.eye(2, dtype=np.float32),
                            np.ones((1, 64), np.float32)).astype(bf16),
        })
    return in_maps


def _run(inputs, trace=False, trace_kwargs=None):
    if "nc" not in _CACHE:
        _CACHE["nc"] = _build()
    nc = _CACHE["nc"]
    in_maps = _prep_inputs(**inputs)
    r = run_bass_kernel_spmd(nc, in_maps, core_ids=list(range(N_CORES)),
                             trace=trace, **(trace_kwargs or {}))
    full = np.empty((B, S, C), np.float32)
    for c in range(N_CORES):
        full[0, 256 * c:256 * (c + 1), :] = r.results[c]["out1"]
        full[1, 256 * c:256 * (c + 1), :] = r.results[c]["out2"]
    return full, r


def kernel(**inputs):
    full, _ = _run(inputs, trace=False)
    return full
